# revision 41
# baseline (speedup 1.0000x reference)
"""GraphNorm-style segmented normalization on 8 Trainium2 NeuronCores.

Strategy (x:[500000,256] f32, batch sorted int, 4096 graphs, params [256]):

- Host: graphs sorted by size (descending), dealt round-robin to 8 cores;
  slot k on every core holds that core's rank-(8k+c) graph, padded to the
  canonical size S_k (rounded to even, then equalized within small
  "stat groups" so one bn_stats instruction covers the whole group).
  Slot structure is identical across cores -> one SPMD Bass program.
- Host packs each core's nodes channel-major and HALF-INTERLEAVED:
  xt[p, 2*w + h] = x[node w, h*128 + p]. bn_stats over [128, m, 2S]
  yields independent (even,odd)=(lo,hi) stats per group member.
- Device (per core, no PE/PSUM): per chunk: DMA load [128, 2W] ->
  grouped bn_stats (DVE) -> batched stats math using
  E[(x-a*mu)^2] = E[x^2] + (a^2-2a)*mu^2 -> rstd via reciprocal+sqrt ->
  per-(slot,half) affine apply out = A*x + B written to an fp16 tile,
  minis balanced across DVE (tensor_scalar), ACT (activation Identity)
  and GPSIMD (tensor_scalar) -> fp16 store (halves write traffic).
- Host un-interleaves, upconverts to f32 and scatters rows back.
"""
import sys

if "/opt/trn_rl_repo" not in sys.path:
    sys.path.insert(0, "/opt/trn_rl_repo")

import numpy as np

import concourse.bacc as bacc
import concourse.tile as tile
from concourse import mybir
from concourse.bass_utils import run_bass_kernel_spmd

F32 = mybir.dt.float32
F16 = mybir.dt.float16
EPS = 1e-9
N_CORES = 8
H = 256
MINI_TGT = 768      # nodes per mini-chunk (DMA/pipeline granule)
SUPER_MINIS = 4     # minis per super-chunk (stats-math batch granule)
X_BUFS = 10         # paired-mini X tiles; ~5 spare pairs of elasticity so
                    # late loads under HBM contention don't cascade into
                    # X-pool starvation and DMA idle gaps
O_BUFS = 8          # per-mini fp16 output tiles in flight (slack so slow
                    # stores under HBM contention don't stall applies)
USE_GPSIMD = True
BN_FMAX = 512       # bn_stats free-size limit
GROUP_CAP = 1       # walrus birverifier requires 6 elem/partition bn_stats
                    # output => no multi-slot bn_stats batching on this
                    # compiler (keep infrastructure; cap=1 disables)
APPLY_LAG = 2       # supers between A/B math and applies (1 starves: sqrt
                    # at ACT queue head stalls applies -> engines idle)
# measured per-op cost models (ns) for the apply split, stream = slot size S
# (fits from v2 HW trace: DVE 315ns, ACT 505ns, GP 496ns @ S=122)
DVE_APPLY_NS = lambda S: 2 * (174 + S) / 0.96
ACT_APPLY_NS = lambda S: 2 * (480 + S) / 1.2
GP_APPLY_NS = lambda S: 2 * (156 + S / 0.36)

_program_cache = {}
_last_run = None


def _plan_slots(sizes, n_cores):
    G = len(sizes)
    Gp = ((G + n_cores - 1) // n_cores) * n_cores
    sizes_p = np.concatenate([sizes, np.zeros(Gp - len(sizes), sizes.dtype)])
    order = np.argsort(-sizes_p, kind="stable")
    ranked = order.reshape(-1, n_cores)
    rank_sz = sizes_p[order].reshape(-1, n_cores)
    S = rank_sz[:, 0]
    keep = S > 0
    ranked = ranked[keep]
    S = S[keep].astype(np.int64)
    S = ((S + 1) // 2) * 2
    return ranked, S


def _plan_groups(S):
    """Group consecutive slots for batched bn_stats; equalize sizes inside
    each group (S is descending so group[0] is the max)."""
    groups = []
    k = 0
    M = len(S)
    while k < M:
        s0 = int(S[k])
        m = 1
        while (k + m < M and m < GROUP_CAP
               and (m + 1) * 2 * s0 <= BN_FMAX):
            m += 1
        groups.append((k, k + m))
        S[k:k + m] = s0
        k += m
    return groups


def _plan_chunks(groups, S, w_tgt):
    """Minis = runs of whole stat-groups totalling ~w_tgt nodes."""
    chunks = []
    cur = []
    acc = 0
    for (g0, g1) in groups:
        cur.append((g0, g1))
        acc += int(S[g0]) * (g1 - g0)
        if acc >= w_tgt:
            chunks.append(cur)
            cur = []
            acc = 0
    if cur:
        chunks.append(cur)
    return chunks


def _plan_supers(minis, super_minis):
    """Full-size supers, but taper the last two supers' worth of minis into
    half-size supers so the drain tail after loads finish is shorter."""
    taper = min(len(minis), 2 * super_minis)
    head, tail = minis[:len(minis) - taper], minis[len(minis) - taper:]
    supers = [head[i:i + super_minis]
              for i in range(0, len(head), super_minis)]
    half = max(1, super_minis // 2)
    supers += [tail[i:i + half] for i in range(0, len(tail), half)]
    return supers


def _build_program(S, offs, supers, M, Np):
    nc = bacc.Bacc("TRN2", target_bir_lowering=False, debug=False,
                   num_devices=N_CORES)
    xt_d = nc.dram_tensor("xt", [128, 2 * Np], F32, kind="ExternalInput")
    # host-folded per-(slot,half) constant tensors (c1 = S/n, caa = a^2-2a):
    #   e2 = c1 * (-w*a)  [so m (x) e2 = -w*a*mu directly]
    #   d2 = c1 + caa*c1^2,  c3 = 1/n
    # plus tiny per-(partition,half) scalars wh = w, bh = b
    e2_d = nc.dram_tensor("e2", [128, M, 2], F32, kind="ExternalInput")
    d2_d = nc.dram_tensor("d2", [128, M, 2], F32, kind="ExternalInput")
    c3_d = nc.dram_tensor("c3", [128, M, 2], F32, kind="ExternalInput")
    wh_d = nc.dram_tensor("wh", [128, 2], F32, kind="ExternalInput")
    bh_d = nc.dram_tensor("bh", [128, 2], F32, kind="ExternalInput")
    yt_d = nc.dram_tensor("yt", [128, 2 * Np], F16, kind="ExternalOutput")

    mult = mybir.AluOpType.mult
    add = mybir.AluOpType.add

    with tile.TileContext(nc) as tc:
        with (
            tc.tile_pool(name="const", bufs=1) as constp,
            tc.tile_pool(name="xp", bufs=X_BUFS) as xp,
            tc.tile_pool(name="op", bufs=O_BUFS) as op_pool,
            # stats/coefficient tiles are tiny (~0.2-0.6KB/partition);
            # deep pools so they never cap the pipeline's run-ahead
            tc.tile_pool(name="stp", bufs=5) as stp,
            tc.tile_pool(name="abp", bufs=5) as abp,
            tc.tile_pool(name="abp3", bufs=5) as abp3,
        ):
            e2t = constp.tile([128, M, 2], F32)
            d2t = constp.tile([128, M, 2], F32)
            c3t = constp.tile([128, M, 2], F32)
            wht = constp.tile([128, 2], F32)
            bht = constp.tile([128, 2], F32)

            def emit_const_loads():
                """Issued AFTER the first super's X loads: 1.6 MB of
                constants would otherwise delay the first bn_stats."""
                nc.sync.dma_start(e2t[:], e2_d[:, :, :])
                nc.sync.dma_start(d2t[:], d2_d[:, :, :])
                nc.sync.dma_start(c3t[:], c3_d[:, :, :])
                nc.sync.dma_start(wht[:], wh_d[:, :])
                nc.sync.dma_start(bht[:], bh_d[:, :])

            v = nc.vector
            # global engine-load accumulators for the 3-way apply balance
            loads = {"dve": 0.0, "act": 0.0, "gp": 0.0}

            def emit_loads(super_):
                """Paired-mini X loads (~18KB per-partition rows sustain a
                better per-DMA-engine rate than 9KB, especially under HBM
                contention). Returns one (X_ap, mk0, mk1) entry per mini."""
                Xs = []
                for pi in range(0, len(super_), 2):
                    pair = super_[pi:pi + 2]
                    p0 = int(offs[pair[0][0][0]])
                    p1 = int(offs[pair[-1][-1][1]])
                    XP = xp.tile([128, 2 * (p1 - p0)], F32, tag="X")
                    nc.sync.dma_start(XP[:], xt_d[:, 2 * p0:2 * p1])
                    for mini in pair:
                        mk0 = mini[0][0]
                        mk1 = mini[-1][1]
                        n0 = int(offs[mk0])
                        n1 = int(offs[mk1])
                        Xs.append((XP[:, 2 * (n0 - p0):2 * (n1 - p0)],
                                   mk0, mk1))
                return Xs

            def emit_front(super_, Xs):
                """Per-slot bn_stats then batched sigma^2 math (DVE)."""
                k0 = super_[0][0][0]
                k1 = super_[-1][-1][1]
                Mc = k1 - k0

                st = stp.tile([128, Mc, 6], F32, tag="st")
                for mini, (X, mk0, mk1) in zip(super_, Xs):
                    n0 = int(offs[mk0])
                    for (g0, g1) in mini:
                        a = int(offs[g0]) - n0
                        s = int(S[g0])
                        m = g1 - g0
                        src = X[:, 2 * a:2 * (a + m * s)]
                        if m > 1:
                            src = src.rearrange("p (m w) -> p m w", m=m)
                        nc.vector.bn_stats(st[:, g0 - k0:g1 - k0, :], src)
                        loads["dve"] += (140 + m * 2 * s) / 0.96

                # interleaved per-(slot,half) fields, [128, 2*Mc] views:
                st_r = st[:].rearrange("p m (x y) -> p (m x) y", x=2, y=3)
                m_v = st_r[:, :, 1]          # means  (lo,hi interleaved)
                v_v = st_r[:, :, 2]          # cnt*var
                e2s = e2t[:, k0:k1, :].rearrange("p m h -> p (m h)")
                d2s = d2t[:, k0:k1, :].rearrange("p m h -> p (m h)")
                c3s = c3t[:, k0:k1, :].rearrange("p m h -> p (m h)")

                U = 2 * Mc
                mu = abp.tile([128, U], F32, tag="mu")
                q = abp.tile([128, U], F32, tag="q")
                ex2 = abp.tile([128, U], F32, tag="ex2")
                sg = abp.tile([128, U], F32, tag="sg")

                # sigma^2 = c1*var_pad + (c1 + caa*c1^2)*m_pad^2 + EPS
                v.tensor_tensor(q[:], m_v, m_v, mult)           # m^2
                v.tensor_tensor(q[:], q[:], d2s, mult)          # *d2
                v.tensor_tensor(ex2[:], v_v, c3s, mult)         # c1*var_pad
                v.scalar_tensor_tensor(sg[:], q[:], EPS, ex2[:],
                                       add, add)                # sigma^2+EPS
                v.tensor_tensor(mu[:], m_v, e2s, mult)          # -w*a*mu
                loads["dve"] += 5 * (82 + U) / 0.96
                return [super_, Xs, mu, sg, None, None, k0]

            def emit_post(ctx):
                """rstd via ACT 1/sqrt|x|, then A/B (DVE) for a front-emitted
                super. Emitted AFTER an older super's applies so the rstd
                never sits at ACT's queue head while DVE runs stats."""
                super_, Xs, mu, sg, _, _, k0 = ctx
                k1 = super_[-1][-1][1]
                U = 2 * (k1 - k0)
                At = abp3.tile([128, U], F32, tag="At")
                Bt = abp3.tile([128, U], F32, tag="Bt")
                nc.scalar.activation(
                    sg[:], sg[:],
                    mybir.ActivationFunctionType.Abs_reciprocal_sqrt)
                loads["act"] += (480 + U) / 1.2
                v.tensor_tensor(Bt[:], mu[:], sg[:], mult)      # -w*a*mu*rstd
                for h in (0, 1):
                    sgh = sg[:].rearrange("p (m h) -> p m h", h=2)[:, :, h]
                    Ah = At[:].rearrange("p (m h) -> p m h", h=2)[:, :, h]
                    Bh = Bt[:].rearrange("p (m h) -> p m h", h=2)[:, :, h]
                    v.tensor_scalar(Ah, sgh, wht[:, h:h + 1], None, mult)
                    v.tensor_scalar(Bh, Bh, 1.0, bht[:, h:h + 1], mult, add)
                loads["dve"] += 5 * (82 + U) / 0.96
                ctx[4] = At
                ctx[5] = Bt
                return ctx

            def emit_applies(ctx):
                """Apply + fp16 store for a super whose A/B math was emitted
                earlier. Each WHOLE mini goes to one engine (a shared output
                tile between engines would serialize them via Tile deps);
                minis are balanced greedily across DVE/ACT/GPSIMD using
                global engine-load accumulators."""
                super_, Xs, _, _, At, Bt, k0 = ctx
                for (X, pk0, pk1) in Xs:
                    n0 = int(offs[pk0])
                    n1 = int(offs[pk1])
                    O = op_pool.tile([128, 2 * (n1 - n0)], F16, tag="O")
                    Xr = X.rearrange("p (w h) -> p w h", h=2)
                    Or = O[:].rearrange("p (w h) -> p w h", h=2)
                    slot_sizes = [int(S[k]) for k in range(pk0, pk1)]
                    cd = sum(DVE_APPLY_NS(s) for s in slot_sizes)
                    ca = sum(ACT_APPLY_NS(s) for s in slot_sizes)
                    cg = sum(GP_APPLY_NS(s) for s in slot_sizes)
                    opts = [("dve", cd), ("act", ca)]
                    if USE_GPSIMD:
                        opts.append(("gp", cg))
                    eng = min(opts, key=lambda ec: loads[ec[0]] + ec[1])[0]
                    loads[eng] += dict(opts)[eng]
                    for k in range(pk0, pk1):
                        a = int(offs[k]) - n0
                        s = int(S[k])
                        for h in (0, 1):
                            j2 = 2 * (k - k0) + h
                            xs = Xr[:, a:a + s, h]
                            os_ = Or[:, a:a + s, h]
                            Ac = At[:, j2:j2 + 1]
                            Bc = Bt[:, j2:j2 + 1]
                            if eng == "dve":
                                v.tensor_scalar(os_, xs, Ac, Bc, mult, add)
                            elif eng == "gp":
                                nc.gpsimd.tensor_scalar(os_, xs, Ac, Bc,
                                                        mult, add)
                            else:
                                nc.scalar.activation(
                                    os_, xs,
                                    mybir.ActivationFunctionType.Identity,
                                    bias=Bc, scale=Ac)
                    nc.sync.dma_start(yt_d[:, 2 * n0:2 * n1], O[:])

            pend = []
            for i, super_ in enumerate(supers):
                Xs = emit_loads(super_)
                if i == 0:
                    emit_const_loads()
                ctx = emit_front(super_, Xs)
                if len(pend) >= APPLY_LAG:
                    emit_applies(pend.pop(0))
                pend.append(emit_post(ctx))
            while pend:
                emit_applies(pend.pop(0))
    nc.compile()
    return nc


def _build_program_cached(S, offs, supers, M, Np):
    key = (tuple(int(s) for s in S),
           tuple(tuple(tuple(g) for g in sup_mini)
                 for sup in supers for sup_mini in sup),
           M, Np)
    nc = _program_cache.get(key)
    if nc is None:
        nc = _build_program(S, offs, supers, M, Np)
        _program_cache[key] = nc
    return nc


def kernel(x, batch, alpha, weight, bias, num_graphs):
    global _last_run
    x = np.asarray(x, dtype=np.float32)
    batch = np.asarray(batch).astype(np.int64)
    alpha = np.asarray(alpha, dtype=np.float32)
    weight = np.asarray(weight, dtype=np.float32)
    bias = np.asarray(bias, dtype=np.float32)
    G = int(num_graphs)
    N, Hx = x.shape
    assert Hx == H

    sizes = np.bincount(batch, minlength=G).astype(np.int64)
    node_order = np.argsort(batch, kind="stable")
    gstarts = np.concatenate([[0], np.cumsum(sizes)])

    ranked, S = _plan_slots(sizes, N_CORES)
    groups = _plan_groups(S)            # equalizes S in-place per group
    offs = np.concatenate([[0], np.cumsum(S)])
    M = len(S)
    Np = int(offs[-1])
    minis = _plan_chunks(groups, S, MINI_TGT)
    supers = _plan_supers(minis, SUPER_MINIS)

    nc = _build_program_cached(S, offs, supers, M, Np)

    caa = alpha * alpha - 2.0 * alpha              # per-channel [256]
    nwa = -(weight * alpha)
    # per-(partition, half) views of the channel params
    caa_ph = np.ascontiguousarray(caa.reshape(2, 128).T)      # [128, 2]
    w_ph = np.ascontiguousarray(weight.reshape(2, 128).T)
    nwa_ph = np.ascontiguousarray(nwa.reshape(2, 128).T)
    b_ph = np.ascontiguousarray(bias.reshape(2, 128).T)

    xa = np.concatenate([x, np.zeros((1, H), np.float32)], axis=0)

    in_maps = []
    idx_per_core = []
    for c in range(N_CORES):
        gids = ranked[:, c]
        n = sizes[gids]
        idx = np.full(Np, N, dtype=np.int64)
        for k in range(M):
            g = gids[k]
            nk = int(n[k])
            if nk:
                idx[int(offs[k]):int(offs[k]) + nk] = \
                    node_order[gstarts[g]:gstarts[g] + nk]
        xp = xa[idx]                                   # [Np, 256]
        # xt[p, 2w+h] = xp[w, h*128+p]
        xv = xp.reshape(Np, 2, 128)
        xt = np.ascontiguousarray(xv.transpose(2, 0, 1)).reshape(128, 2 * Np)
        nguard = np.maximum(n, 1).astype(np.float32)
        c1 = (S.astype(np.float32) / nguard)               # [M]
        c3 = (1.0 / nguard)
        # [128, M, 2] per-(partition, slot, half) constant tensors
        # e2 = c1 * (-w*a): m (x) e2 gives -w*a*mu in one op
        e2b = (c1[None, :, None] * nwa_ph[:, None, :]).astype(np.float32)
        # d2 = c1 + caa*c1^2   (caa varies per channel -> partition x half)
        d2b = (c1[None, :, None] +
               caa_ph[:, None, :] * (c1 * c1)[None, :, None]).astype(
            np.float32)
        c3b = np.broadcast_to(c3[None, :, None], (128, M, 2)).astype(
            np.float32).copy()
        in_maps.append({
            "xt": xt, "e2": e2b, "d2": d2b, "c3": c3b,
            "wh": w_ph, "bh": b_ph,
        })
        idx_per_core.append(idx)
    del xa

    _last_run = (nc, in_maps)
    res = run_bass_kernel_spmd(nc, in_maps, core_ids=list(range(N_CORES)))

    out = np.empty((N, H), dtype=np.float32)
    for c in range(N_CORES):
        yt = np.asarray(res.results[c]["yt"])          # [128, 2*Np] fp16
        yv = yt.reshape(128, Np, 2)
        # out_packed[w, h*128+p] = yv[p, w, h]
        yp = np.ascontiguousarray(
            yv.transpose(1, 2, 0)).reshape(Np, H).astype(np.float32)
        idx = idx_per_core[c]
        mask = idx < N
        out[idx[mask]] = yp[mask]
    return out
